# revision 29
# baseline (speedup 1.0000x reference)
"""Trainium2 Bass kernel for a dense transformer block (LN1 -> MHA -> LN2 -> MLP).

Sharding: 8 cores = (batch b in 0..3) x (sequence half in 0..1), zero
cross-core communication. Each core's input tokens are reordered on the host
so its 1024 query tokens are always tokens 0..1023 of its 2048-token view
(key/value order is irrelevant to attention), letting one SPMD program serve
every core and the query-side LN reuse the full-sequence LN output.

Precision: fp8e4m3 DoubleRow matmuls for QKV/O projections, ctx, and the MLP
(weights pre-scaled by power-of-2 factors on the host; descales fold into
existing bias/scale stages, so they cost nothing). Scores stay bf16.
LayerNorm gain/bias are folded into the following weights on the host
(mathematically exact), so the device LN is a pure (x-mu)*rstd normalize.

Softmax: exp(score - C) with a host-estimated shift C keeping exp outputs in
fp8 range; the denominator is produced by a ones-column appended to V inside
the ctx DoubleRow matmul (out partition 65), so it costs no extra PE time.

Schedule: attention for the first head-pairs is emitted between the two
projection groups so the Act engine's exp stream starts as early as possible;
the MLP of each query half is chunk-interleaved into the next half's
attention so gelus stay clustered (minimizing activation-table reloads) while
PE fills Act-bound stretches. LN2's rstd is computed entirely on DVE
(reciprocal_approx_fast + Newton) to avoid sqrt-table loads mid-stream.
"""

import math
import sys

if '/opt/trn_rl_repo' not in sys.path:
    sys.path.insert(0, '/opt/trn_rl_repo')

import numpy as np
import ml_dtypes

import concourse.tile as tile
import concourse.mybir as mybir
from concourse import bacc
from concourse.bass import ts
from concourse.bass_utils import run_bass_kernel_spmd

P = 128
F32 = mybir.dt.float32
F32R = mybir.dt.float32r
BF16 = mybir.dt.bfloat16
F8 = mybir.dt.float8e4
AF = mybir.ActivationFunctionType
DR = mybir.MatmulPerfMode.DoubleRow
ALU = mybir.AluOpType
EPS = 1e-6

B, S, D, H, MLP = 4, 2048, 1024, 16, 4096
N_CORES = 8
NP_F8 = ml_dtypes.float8_e4m3


def build_bass(T, Q, Dm, Hh, Mlp, n_cores, scales, dbg=False):
    s_wq, s_wk, s_wv, s_wo, s_w1, s_w2, shift_c = scales
    dh = Dm // Hh
    assert dh == 64
    n_dc = Dm // P          # 8 feature chunks
    n_cj = n_dc // 2        # 4 DoubleRow k-pair steps over D
    n_tk = T // P           # 16 token chunks
    TB = 512
    n_tb = T // TB          # 4
    QQ = 512
    n_qq = Q // QQ          # 2
    n_mo = Mlp // P         # 32
    n_m2 = n_mo // 2        # 16 DoubleRow k-pair steps over MLP
    n_hp = Hh // 2          # 8 head pairs
    inv_d = 1.0 / Dm
    exp_scale = 0.125 / (s_wq * s_wk)
    c_wo = 1.0 / (s_wo * s_wv)
    inv_s1 = 1.0 / s_w1
    inv_s2 = 1.0 / s_w2

    nc = bacc.Bacc("TRN2", target_bir_lowering=False, debug=False,
                   enable_asserts=False, num_devices=n_cores)

    def din(name, shape, dt):
        return nc.dram_tensor(name, shape, dt, kind="ExternalInput").ap()

    xT_d = din("xT", (Dm, T), F32)
    wq_d, wk_d = din("wq8", (Dm, Dm), F8), din("wk8", (Dm, Dm), F8)
    wv_d, wo_d = din("wv8", (Dm, Dm), F8), din("wo8", (Dm, Dm), F8)
    w1_d = din("w18", (Dm, Mlp), F8)
    w2_d = din("w28", (2, Mlp, Dm), F8)
    bq_d, bk_d = din("bq", (Dm,), F32), din("bk", (Dm,), F32)
    bv_d, bo_d = din("bv16", (Dm,), BF16), din("bo", (Dm,), F32)
    b1_d, b2_d = din("b1", (Mlp,), F32), din("b2", (Dm,), F32)
    ones_d = din("ones32", (P, P), F32)
    yT_d = nc.dram_tensor("yT", (Dm, Q), F32, kind="ExternalOutput").ap()
    dbg_d = {}
    if dbg:
        for nm, shape, dt in [("dXN8", (Dm, T), F8), ("dKT", (Dm, T), BF16),
                              ("dQT", (Dm, Q), BF16),
                              ("dVT", (P, 16 * 16 * 65), F8),
                              ("dCT", (Dm, Q), F8), ("dXQ", (Dm, Q), BF16),
                              ("dXN2", (Dm, 2 * Q), F8),
                              ("dY1", (Mlp, 2 * 512), F8),
                              ("dEXP", (P, 16 * 512), F8),
                              ("dPSC", (65, 512), F32)]:
            dbg_d[nm] = nc.dram_tensor(nm, shape, dt,
                                       kind="ExternalOutput").ap()

    with tile.TileContext(nc) as tc, \
         tc.tile_pool(name="const", bufs=1) as constp, \
         tc.tile_pool(name="p_res", bufs=1) as p_res, \
         tc.tile_pool(name="p_kv", bufs=1) as p_kv, \
         tc.tile_pool(name="p_exp", bufs=2) as p_exp, \
         tc.tile_pool(name="p_rb", bufs=1) as p_rb, \
         tc.tile_pool(name="p_ct", bufs=1) as p_ct, \
         tc.tile_pool(name="ps_sc", bufs=2, space="PSUM") as ps_sc, \
         tc.tile_pool(name="ps_ctx", bufs=2, space="PSUM") as ps_ctx, \
         tc.tile_pool(name="ps_sh", bufs=2, space="PSUM") as ps_sh:

        ones_fr = constp.tile([P, P], F32R)
        nc.sync.dma_start(ones_fr[:], ones_d[:, :].bitcast(F32R))
        ones_f = constp.tile([P, P], BF16)
        nc.vector.memset(ones_f[:], 1.0)
        eps_t = constp.tile([P, 1], F32)
        nc.vector.memset(eps_t[:], EPS)
        negc_t = constp.tile([P, 1], F32)
        nc.vector.memset(negc_t[:], -shift_c)

        def vec_tile(src, n, nm):
            t = constp.tile([P, n], F32, tag=nm, name=nm)
            nc.sync.dma_start(t[:], src.rearrange("(c p) -> p c", p=P))
            return t

        bq_t, bk_t = vec_tile(bq_d, n_dc, "bq"), vec_tile(bk_d, n_dc, "bk")
        bo_t, b2_t = vec_tile(bo_d, n_dc, "bo"), vec_tile(b2_d, n_dc, "b2")
        b1_t = vec_tile(b1_d, n_mo, "b1")

        XQ = p_res.tile([P, n_dc, Q], BF16)       # residual stream (bf16)
        KT = p_kv.tile([P, n_dc, T], BF16)
        QT = p_kv.tile([P, n_dc, Q], BF16)
        VT = p_kv.tile([P, n_tk, Hh, 65], F8)
        nc.gpsimd.memset(VT[:, :, :, 64:65], 1.0)
        CT8 = p_ct.tile([P, n_dc, Q], F8)

        def attn_block(qq, hp):
            qsl = ts(qq, QQ)
            exps = [p_exp.tile([P, n_tk, QQ], F8, tag="expT", name="expT")
                    for _ in range(2)]
            for kcp in range(n_tk // 2):
                for hi in range(2):
                    r0 = hi * 64
                    pss = ps_sc.tile([P, 2, QQ], F32, tag="ps_s", name="ps_s")
                    for j in range(2):
                        nc.tensor.matmul(
                            pss[:, j, :],
                            KT[r0:r0 + 64, hp, ts(2 * kcp + j, P)],
                            QT[r0:r0 + 64, hp, qsl],
                            start=True, stop=True)
                    nc.scalar.activation(
                        exps[hi][:, 2 * kcp:2 * kcp + 2, :],
                        pss[:, :, :], AF.Exp,
                        scale=exp_scale, bias=negc_t[:, 0:1])
            pcs = [ps_ctx.tile([65, QQ], F32, tag="ps_c", name="ps_c")
                   for _ in range(2)]
            for kcp in range(n_tk // 2):
                for hi in range(2):
                    h = 2 * hp + hi
                    nc.tensor.matmul(
                        pcs[hi][:, :],
                        VT[:, 2 * kcp:2 * kcp + 2, h, 0:65],
                        exps[hi][:, 2 * kcp:2 * kcp + 2, :],
                        start=(kcp == 0), stop=(kcp == n_tk // 2 - 1),
                        perf_mode=DR)
            if dbg and qq == 0 and hp == 0:
                nc.sync.dma_start(
                    dbg_d["dEXP"][:, :],
                    exps[0][:].rearrange("p a b -> p (a b)"))
                psc_sb = p_exp.tile([65, QQ], F32, tag="expT", name="expT")
                nc.vector.tensor_copy(psc_sb[:], pcs[0][:, :])
                nc.sync.dma_start(dbg_d["dPSC"][:, :], psc_sb[:])
            for hi in range(2):
                r0 = hi * 64
                rb1 = p_rb.tile([1, QQ], F32, tag="rb1")
                nc.vector.reciprocal(rb1[:], pcs[hi][64:65, :])
                rbb = p_rb.tile([64, QQ], F32, tag="rbb")
                nc.gpsimd.partition_broadcast(rbb[:], rb1[:])
                nc.vector.tensor_mul(CT8[r0:r0 + 64, hp, qsl],
                                     pcs[hi][0:64, :], rbb[:])

        # ================= Phase A: LN1 + projections (+early attn) ========
        with tc.tile_pool(name="p_xn8", bufs=1) as p_xn8, \
             tc.tile_pool(name="p_x", bufs=2) as p_x, \
             tc.tile_pool(name="p_t", bufs=2) as p_t, \
             tc.tile_pool(name="p_w", bufs=3) as p_w, \
             tc.tile_pool(name="p_wv", bufs=2) as p_wv:

            XN8 = p_xn8.tile([P, n_dc, T], F8)
            bv_row = p_xn8.tile([1, Dm], BF16)
            nc.sync.dma_start(bv_row[:, :], bv_d[None, :])
            bv_bc = p_xn8.tile([P, Dm], BF16)
            nc.gpsimd.partition_broadcast(bv_bc[:], bv_row[:])

            for tb in range(n_tb):
                tsl = ts(tb, TB)
                xc = p_x.tile([P, n_dc, TB], F32R, tag="xc")
                for dc in range(n_dc):
                    nc.sync.dma_start(xc[:, dc, :],
                                      xT_d[ts(dc, P), tsl].bitcast(F32R))
                st = ps_sc.tile([P, 2, TB], F32, tag="ps_s", name="ps_s")
                for dc in range(n_dc):
                    nc.tensor.matmul(st[:, 0, :], ones_fr[:], xc[:, dc, :],
                                     start=(dc == 0), stop=(dc == n_dc - 1))
                    xsq = p_t.tile([P, TB], BF16, tag="xsq")
                    nc.scalar.activation(xsq[:], xc[:, dc, :].bitcast(F32),
                                         AF.Square)
                    nc.tensor.matmul(st[:, 1, :], ones_f[:], xsq[:],
                                     start=(dc == 0), stop=(dc == n_dc - 1))
                    if tb < Q // TB:
                        nc.vector.tensor_copy(XQ[:, dc, tsl],
                                              xc[:, dc, :].bitcast(F32))
                mbc = p_t.tile([P, TB], F32, tag="mbc")
                nc.vector.tensor_scalar_mul(mbc[:], st[:, 0, :], inv_d)
                var = p_t.tile([P, TB], F32, tag="var")
                nc.vector.tensor_scalar(var[:], st[:, 1, :], inv_d, EPS,
                                        op0=ALU.mult, op1=ALU.add)
                m2 = p_t.tile([P, TB], F32, tag="tn")
                nc.vector.tensor_mul(m2[:], mbc[:], mbc[:])
                nc.vector.tensor_sub(var[:], var[:], m2[:])
                std = p_t.tile([P, TB], F32, tag="stdt")
                nc.scalar.activation(std[:], var[:], AF.Sqrt)
                rstd = p_t.tile([P, TB], F32, tag="rstd")
                nc.vector.reciprocal(rstd[:], std[:])
                for dc in range(n_dc):
                    t0 = p_t.tile([P, TB], F32, tag="tn")
                    nc.vector.tensor_sub(t0[:], xc[:, dc, :].bitcast(F32),
                                         mbc[:])
                    nc.gpsimd.tensor_mul(XN8[:, dc, tsl], t0[:], rstd[:])

            def q_proj(mo):
                wt = p_w.tile([P, n_cj, 2, P], F8, tag="wq")
                nc.sync.dma_start(
                    wt[:],
                    wq_d[:, ts(mo, P)]
                    .rearrange("(c j p) m -> p c j m", j=2, p=P))
                for qb in range(n_qq):
                    ps = ps_sh.tile([P, QQ], F32, tag="sh")
                    for cj in range(n_cj):
                        nc.tensor.matmul(
                            ps[:], wt[:, cj, :, :],
                            XN8[:, 2 * cj:2 * cj + 2, ts(qb, QQ)],
                            start=(cj == 0), stop=(cj == n_cj - 1),
                            perf_mode=DR)
                    nc.vector.tensor_scalar_add(QT[:, mo, ts(qb, QQ)], ps[:],
                                                bq_t[:, mo:mo + 1])

            def k_proj(mo):
                wt = p_w.tile([P, n_cj, 2, P], F8, tag="wq")
                nc.sync.dma_start(
                    wt[:],
                    wk_d[:, ts(mo, P)]
                    .rearrange("(c j p) m -> p c j m", j=2, p=P))
                for tb in range(n_tb):
                    ps = ps_sh.tile([P, TB], F32, tag="sh")
                    for cj in range(n_cj):
                        nc.tensor.matmul(
                            ps[:], wt[:, cj, :, :],
                            XN8[:, 2 * cj:2 * cj + 2, ts(tb, TB)],
                            start=(cj == 0), stop=(cj == n_cj - 1),
                            perf_mode=DR)
                    nc.vector.tensor_scalar_add(KT[:, mo, ts(tb, TB)], ps[:],
                                                bk_t[:, mo:mo + 1])

            def v_proj(no):
                NO = 512
                wt = p_wv.tile([P, n_cj, 2, NO], F8, tag="wv")
                nc.sync.dma_start(
                    wt[:],
                    wv_d[:, ts(no, NO)]
                    .rearrange("(c j p) m -> p c j m", j=2, p=P))
                for to in range(n_tk):
                    ps = ps_sh.tile([P, NO], F32, tag="sh")
                    for cj in range(n_cj):
                        nc.tensor.matmul(
                            ps[:], XN8[:, 2 * cj:2 * cj + 2, ts(to, P)],
                            wt[:, cj, :, :],
                            start=(cj == 0), stop=(cj == n_cj - 1),
                            perf_mode=DR)
                    nc.vector.tensor_add(VT[:, to, 8 * no:8 * no + 8, 0:64],
                                         ps[:], bv_bc[:, ts(no, NO)])

            for mo in range(4):
                k_proj(mo)
                q_proj(mo)
            v_proj(0)
            attn_block(0, 0)
            attn_block(0, 1)
            k_proj(4)
            q_proj(4)
            attn_block(0, 2)
            k_proj(5)
            q_proj(5)
            attn_block(0, 3)
            for mo in range(6, n_dc):
                k_proj(mo)
                q_proj(mo)
            v_proj(1)

        # ================= Phase B: rest of attention + Wo + MLP ===========
        with tc.tile_pool(name="p_mlp", bufs=1) as p_mlp, \
             tc.tile_pool(name="p_t2", bufs=2) as p_t2, \
             tc.tile_pool(name="p_t2s", bufs=1) as p_t2s, \
             tc.tile_pool(name="p_wos", bufs=2) as p_wos, \
             tc.tile_pool(name="p_w1", bufs=2) as p_w1, \
             tc.tile_pool(name="p_w2", bufs=2) as p_w2, \
             tc.tile_pool(name="p_y1", bufs=1) as p_y1, \
             tc.tile_pool(name="p_out", bufs=1) as p_out:

            XN2 = p_mlp.tile([P, n_dc, 2, Q], F8)  # [.., hi/lo, ..]
            y1s = {}

            def wo_block(qq):
                qsl = ts(qq, QQ)
                wt_all = None
                for mo in range(n_dc):
                    if mo % 4 == 0:
                        wt_all = p_wos.tile([P, n_cj, 2, 4, P], F8, tag="wo")
                        nc.sync.dma_start(
                            wt_all[:],
                            wo_d[:, ts(mo // 4, 4 * P)]
                            .rearrange("(c j p) (mo m) -> p c j mo m",
                                       j=2, p=P, m=P))
                    ps = ps_sh.tile([P, QQ], F32, tag="sh")
                    for cj in range(n_cj):
                        nc.tensor.matmul(
                            ps[:], wt_all[:, cj, :, mo % 4, :],
                            CT8[:, 2 * cj:2 * cj + 2, qsl],
                            start=(cj == 0), stop=(cj == n_cj - 1),
                            perf_mode=DR)
                    tw = p_out.tile([P, QQ], F32, tag="ot")
                    nc.vector.tensor_scalar(tw[:], ps[:], c_wo,
                                            bo_t[:, mo:mo + 1],
                                            op0=ALU.mult, op1=ALU.add)
                    nc.vector.tensor_add(XQ[:, mo, qsl], tw[:],
                                         XQ[:, mo, qsl])

            def ln2_block(qq):
                qsl = ts(qq, QQ)
                st2 = ps_sc.tile([P, 2, QQ], F32, tag="ps_s", name="ps_s")
                for dc in range(n_dc):
                    nc.tensor.matmul(st2[:, 0, :], ones_f[:], XQ[:, dc, qsl],
                                     start=(dc == 0), stop=(dc == n_dc - 1))
                    sq = p_t2.tile([P, QQ], BF16, tag="sq2")
                    nc.gpsimd.tensor_mul(sq[:], XQ[:, dc, qsl],
                                         XQ[:, dc, qsl])
                    nc.tensor.matmul(st2[:, 1, :], ones_f[:], sq[:],
                                     start=(dc == 0), stop=(dc == n_dc - 1))
                mbc = p_t2s.tile([P, QQ], F32, tag="mbc2")
                nc.vector.tensor_scalar_mul(mbc[:], st2[:, 0, :], inv_d)
                var = p_t2s.tile([P, QQ], F32, tag="var2")
                nc.vector.tensor_scalar(var[:], st2[:, 1, :], inv_d, EPS,
                                        op0=ALU.mult, op1=ALU.add)
                m2 = p_t2.tile([P, QQ], F32, tag="tn2")
                nc.vector.tensor_mul(m2[:], mbc[:], mbc[:])
                nc.vector.tensor_sub(var[:], var[:], m2[:])
                # rstd = rsqrt(var) on DVE only: seed from 1/var + Newton
                r = p_t2s.tile([P, QQ], F32, tag="rstd2")
                nc.vector.reciprocal_approx_fast(r[:], var[:])
                nc.vector.tensor_scalar(r[:], r[:], 0.72, 0.35,
                                        op0=ALU.mult, op1=ALU.add)
                for _ in range(2):
                    t1 = p_t2.tile([P, QQ], F32, tag="tn2")
                    nc.vector.tensor_mul(t1[:], r[:], r[:])
                    nc.vector.tensor_mul(t1[:], t1[:], var[:])
                    nc.vector.tensor_scalar(t1[:], t1[:], -0.5, 1.5,
                                            op0=ALU.mult, op1=ALU.add)
                    nc.vector.tensor_mul(r[:], r[:], t1[:])
                for dc in range(n_dc):
                    t0 = p_t2.tile([P, QQ], F32, tag="tn2")
                    nc.vector.tensor_sub(t0[:], XQ[:, dc, qsl], mbc[:])
                    m = p_t2.tile([P, QQ], F32, tag="m32")
                    nc.vector.tensor_mul(m[:], t0[:], r[:])
                    nc.vector.tensor_copy(XN2[:, dc, 0, qsl], m[:])
                    nc.vector.tensor_sub(XN2[:, dc, 1, qsl], m[:],
                                         XN2[:, dc, 0, qsl])

            def fc1_block(qq, mo0, mo1):
                qsl = ts(qq, QQ)
                if qq not in y1s:
                    y1s[qq] = p_y1.tile([P, n_mo, 2, QQ], F8, tag="y1",
                                        name="y1")
                Y1 = y1s[qq]
                for mo in range(mo0, mo1):
                    if mo % 4 == 0:
                        wt8 = p_w1.tile([P, n_cj, 2, 4, P], F8, tag="w1")
                        nc.sync.dma_start(
                            wt8[:],
                            w1_d[:, ts(mo // 4, 4 * P)]
                            .rearrange("(c j p) (mo m) -> p c j mo m",
                                       j=2, p=P, m=P))
                    ps = ps_sh.tile([P, QQ], F32, tag="sh")
                    for lv in range(2):
                        for cj in range(n_cj):
                            nc.tensor.matmul(
                                ps[:], wt8[:, cj, :, mo % 4, :],
                                XN2[:, 2 * cj:2 * cj + 2, lv, qsl],
                                start=(lv == 0 and cj == 0),
                                stop=(lv == 1 and cj == n_cj - 1),
                                perf_mode=DR)
                    g32 = p_t2.tile([P, QQ], F32, tag="m32")
                    nc.scalar.activation(g32[:], ps[:], AF.Gelu,
                                         bias=b1_t[:, mo:mo + 1],
                                         scale=inv_s1)
                    nc.vector.tensor_copy(Y1[:, mo, 0, :], g32[:])
                    nc.vector.tensor_sub(Y1[:, mo, 1, :], g32[:],
                                         Y1[:, mo, 0, :])

            def fc2_block(qq):
                qsl = ts(qq, QQ)
                Y1 = y1s.pop(qq)
                for mo2 in range(n_dc):
                    wth = p_w2.tile([P, n_m2, 2, P], F8, tag="w2h")
                    nc.sync.dma_start(
                        wth[:],
                        w2_d[0, :, ts(mo2, P)]
                        .rearrange("(c j p) m -> p c j m", j=2, p=P))
                    wtl = p_w2.tile([P, n_m2, 2, P], F8, tag="w2l")
                    nc.sync.dma_start(
                        wtl[:],
                        w2_d[1, :, ts(mo2, P)]
                        .rearrange("(c j p) m -> p c j m", j=2, p=P))
                    ps = ps_sh.tile([P, QQ], F32, tag="sh")
                    terms = [(wth, 0), (wth, 1), (wtl, 0)]
                    for ti, (wt, lv) in enumerate(terms):
                        for cj in range(n_m2):
                            nc.tensor.matmul(
                                ps[:], wt[:, cj, :, :],
                                Y1[:, 2 * cj:2 * cj + 2, lv, :],
                                start=(ti == 0 and cj == 0),
                                stop=(ti == 2 and cj == n_m2 - 1),
                                perf_mode=DR)
                    ot = p_out.tile([P, QQ], F32, tag="ot")
                    nc.vector.tensor_scalar(ot[:], ps[:], inv_s2,
                                            b2_t[:, mo2:mo2 + 1],
                                            op0=ALU.mult, op1=ALU.add)
                    nc.vector.tensor_add(ot[:], ot[:], XQ[:, mo2, qsl])
                    nc.sync.dma_start(yT_d[ts(mo2, P), qsl], ot[:])

            for hp in range(4, n_hp):
                attn_block(0, hp)
            if dbg:
                for dc in range(n_dc):
                    nc.sync.dma_start(dbg_d["dXN8"][ts(dc, P), :], XN8[:, dc, :])
                    nc.sync.dma_start(dbg_d["dKT"][ts(dc, P), :], KT[:, dc, :])
                    nc.sync.dma_start(dbg_d["dQT"][ts(dc, P), :], QT[:, dc, :])
                    nc.sync.dma_start(dbg_d["dCT"][ts(dc, P), :], CT8[:, dc, :])
                nc.sync.dma_start(dbg_d["dVT"][:, :],
                                  VT[:].rearrange("p a b c -> p (a b c)"))
            wo_block(0)
            if dbg:
                for dc in range(n_dc):
                    nc.sync.dma_start(dbg_d["dXQ"][ts(dc, P), 0:QQ],
                                      XQ[:, dc, 0:QQ])
            attn_block(1, 0)
            attn_block(1, 1)
            ln2_block(0)
            if dbg:
                for dc in range(n_dc):
                    nc.sync.dma_start(
                        dbg_d["dXN2"][ts(dc, P), :],
                        XN2[:, dc, :, :].rearrange("p a b -> p (a b)"))
            for hp in range(2, n_hp):
                attn_block(1, hp)
            wo_block(1)
            fc1_block(0, 0, n_mo)
            if dbg:
                for mo in range(n_mo):
                    nc.sync.dma_start(
                        dbg_d["dY1"][ts(mo, P), :],
                        y1s[0][:, mo, :, :].rearrange("p a b -> p (a b)"))
            ln2_block(1)
            fc2_block(0)
            fc1_block(1, 0, n_mo)
            fc2_block(1)
    nc.compile()
    return nc


_NC_CACHE = {}


def _get_nc(T, Q, Dm, Hh, Mlp, n_cores,
            scales=(16.0, 16.0, 16.0, 16.0, 16.0, 16.0, 3.5)):
    key = (T, Q, Dm, Hh, Mlp, n_cores, tuple(scales))
    if key not in _NC_CACHE:
        _NC_CACHE[key] = build_bass(T, Q, Dm, Hh, Mlp, n_cores, scales)
    return _NC_CACHE[key]


def _split_f8(w):
    hi = w.astype(NP_F8)
    lo = (w - hi.astype(np.float32)).astype(NP_F8)
    return np.stack([hi, lo])


def _pow2_scale(absmax, target=128.0):
    a = float(absmax)
    if not np.isfinite(a) or a <= 0:
        return 1.0
    return float(2.0 ** math.floor(math.log2(target / a)))


def prepare(inputs):
    """Host-side prep: LN folding, fp8 quantization, per-core input maps."""
    f = lambda k: np.asarray(inputs[k], np.float32)
    x = f("x")
    Bq, Sq, Dq = x.shape
    Qtok = Sq // 2
    g1, b1ln = f("ln1_g"), f("ln1_b")
    g2, b2ln = f("ln2_g"), f("ln2_b")
    Wq, Wk, Wv, Wo = f("Wq"), f("Wk"), f("Wv"), f("Wo")
    W1, W2 = f("W1"), f("W2")
    bq, bk, bv, bo = f("bq"), f("bk"), f("bv"), f("bo")
    b1, b2 = f("b1"), f("b2")

    # fold LN1 gain/bias into QKV, LN2 gain/bias into W1 (exact)
    Wq_e = g1[:, None] * Wq
    Wk_e = g1[:, None] * Wk
    Wv_e = g1[:, None] * Wv
    bq_e = bq + b1ln @ Wq
    bk_e = bk + b1ln @ Wk
    bv_e = bv + b1ln @ Wv
    W1_e = g2[:, None] * W1
    b1_e = b1 + b2ln @ W1

    s_wq = _pow2_scale(np.abs(Wq_e).max())
    s_wk = _pow2_scale(np.abs(Wk_e).max())
    # V result is stored in fp8 still scaled by s_wv: bound both weight and
    # activation range (sigma of v_j ~ col norm of Wv_e, x is LN'd)
    vcol = np.sqrt((Wv_e ** 2).sum(0))
    vmag = max(float((vcol * 8).max()), float(np.abs(bv_e).max() * 4), 1e-6)
    s_wv = min(_pow2_scale(np.abs(Wv_e).max()),
               _pow2_scale(vmag, target=200.0))
    s_wo = _pow2_scale(np.abs(Wo).max())
    s_w1 = _pow2_scale(np.abs(W1_e).max())
    s_w2 = _pow2_scale(np.abs(W2).max())

    # estimate max attention score for the exp shift C (sampled)
    mu = x.mean(-1, keepdims=True)
    va = x.var(-1, keepdims=True)
    xn_h = (x - mu) / np.sqrt(va + EPS)
    qi = xn_h[:, ::89][:, :16].reshape(-1, Dq)
    ki = xn_h[:, ::13][:, :128].reshape(-1, Dq)
    qp = (qi @ Wq_e + bq_e).reshape(Bq, -1, H, Dq // H)
    kp = (ki @ Wk_e + bk_e).reshape(Bq, -1, H, Dq // H)
    sc = np.einsum("bqhd,bkhd->bhqk", qp, kp) / np.sqrt(Dq // H)
    shift_c = float(sc.max() + 2.0 * sc.std() - math.log(32.0))

    scales = (s_wq, s_wk, s_wv, s_wo, s_w1, s_w2, shift_c)
    nc = _get_nc(Sq, Qtok, Dq, H, MLP, N_CORES, scales)

    shared = {
        "wq8": (Wq_e * s_wq).astype(NP_F8),
        "wk8": (Wk_e * s_wk).astype(NP_F8),
        "wv8": (Wv_e * s_wv).astype(NP_F8),
        "wo8": (Wo * s_wo).astype(NP_F8),
        "w18": (W1_e * s_w1).astype(NP_F8),
        "w28": _split_f8(W2 * s_w2),
        "bq": (bq_e * s_wq).astype(np.float32),
        "bk": (bk_e * s_wk).astype(np.float32),
        "bv16": (bv_e * s_wv).astype(ml_dtypes.bfloat16),
        "bo": bo.astype(np.float32),
        "b1": b1_e.astype(np.float32),
        "b2": b2.astype(np.float32),
        "ones32": np.ones((P, P), np.float32),
    }
    in_maps = []
    for c in range(N_CORES):
        b = c // 2
        half = c % 2
        xb = x[b]
        xr = np.concatenate(
            [xb[half * Qtok:(half + 1) * Qtok],
             xb[(1 - half) * Qtok:(2 - half) * Qtok]], axis=0)
        m = dict(shared)
        m["xT"] = np.ascontiguousarray(xr.T)
        in_maps.append(m)
    return nc, in_maps, Qtok


def unshard(res, Bq, Sq, Dq, Qtok):
    out = np.empty((Bq, Sq, Dq), np.float32)
    for c in range(N_CORES):
        b = c // 2
        half = c % 2
        out[b, half * Qtok:(half + 1) * Qtok, :] = res.results[c]["yT"].T
    return out


def kernel(**inputs):
    x = np.asarray(inputs["x"], np.float32)
    Bq, Sq, Dq = x.shape
    nc, in_maps, Qtok = prepare(inputs)
    res = run_bass_kernel_spmd(nc, in_maps, core_ids=list(range(N_CORES)))
    return unshard(res, Bq, Sq, Dq, Qtok)
